# revision 15
# baseline (speedup 1.0000x reference)
"""Trainium2 Bass kernel for graph-contrastive loss (nn_PrePrompt_75496935129282).

Computation (reference):
    self = segment_sum(logits_origin, ori_idx, G)       # [G, D]
    pos  = segment_sum(logits_pos,  pos_idx, G)         # [G, D]
    sim[g, k]  = cos(self[g], pos[k])   (eps-guarded norms)
    res[g] = log(sum_s exp(sim[g, neg_idx[g, s]])) - sim[g, g]
    out = mean(res)

Device strategy (8 NeuronCores, SPMD):
  - Nodes sharded 8 ways. Host orders each core's nodes by graph block
    (gid >> 7, 16 blocks of 128) with data-driven per-block chunk
    counts: every 128-node chunk targets one block, so the one-hot
    matmul is [128, 128] per chunk. Blocks are processed EVENS FIRST
    so PSUM/stage halves split by block parity.
  - Streams are fp8e4m3; chunk PAIRS go through one DoubleRow matmul
    (256 nodes per PE pass at 0.5 cycles/row); odd tails use a single
    fp8 matmul. One-hots for 16 chunks are generated with a single
    broadcast is_equal against a block-relative bf16 iota.
  - Exchange (no ring collectives): per-parity AllToAll redistributes
    raw per-core partials (core r receives all 8 cores' partials of
    blocks {2r, 2r+1}); the 8-way sum happens locally (tree adds, f32)
    on gpsimd (pos, hidden under the origin phase) / vector (self).
    Each core normalizes + PE-transposes only its OWN 2 pos blocks and
    a single shared-output AllGather of the 64KB pn_T slice rebuilds
    the full [128d, 2, 2048] column table on every core.
  - Tail: cosine Gram of the core's 256 self rows vs all 2048 columns,
    denominator = exp-accumulate of (sim + ln(count)) with a
    host-precomputed f32 count table, numerator from the raw f32 local
    sums scaled by inverse norms. Per-core losses summed on host.
"""

import os
import sys

sys.path.insert(0, "/opt/trn_rl_repo")

import numpy as np

import concourse.bacc as bacc
import concourse.bass as bass  # noqa: F401
import concourse.mybir as mybir
import concourse.tile as tile
from concourse import bass_isa
from concourse.bass_utils import run_bass_kernel_spmd


def _ensure_ntff_hook():
    """The agent image's antenv lacks axon_hooks; inject it and register
    the ctypes NTFF profiling hook so trace=True works under axon."""
    import types

    import antenv

    if hasattr(antenv, "axon_hooks"):
        return
    mod = types.ModuleType("antenv.axon_hooks")
    mod._hook = None

    def set_axon_ntff_profile_hook(h):
        mod._hook = h

    def get_axon_ntff_profile_hook():
        return mod._hook

    mod.set_axon_ntff_profile_hook = set_axon_ntff_profile_hook
    mod.get_axon_ntff_profile_hook = get_axon_ntff_profile_hook
    sys.modules["antenv.axon_hooks"] = mod
    antenv.axon_hooks = mod
    try:
        from trn_agent_boot.trn_boot import _ntff_profile_via_ctypes

        mod._hook = _ntff_profile_via_ctypes("/opt/axon/libaxon_pjrt.so")
    except Exception as e:  # pragma: no cover
        print(f"ntff hook registration failed: {e}")


F32 = mybir.dt.float32
BF16 = mybir.dt.bfloat16
F8 = mybir.dt.float8e4

G = 2048
S = 127
D = 256
NCORES = 8
P = 128
A = 16  # chunk-count quantum (padding granularity)
AW = 32  # chunks per DMA group (packet size = AW*D fp8 per partition)
NBUK = 16  # graph blocks of 128
GLOC = G // NCORES  # 256

# even blocks first, then odd: PSUM generation k holds blocks ORDER[8k:8k+8]
ORDER = list(range(0, NBUK, 2)) + list(range(1, NBUK, 2))

_MM_RAW = os.environ.get("KERNEL_MM_DT", "f8")
MMDT = BF16 if _MM_RAW == "bf16" else F8


def _chunk_groups(nchunk):
    """DMA groups: small warm-up groups (fast pipeline start), then
    AW-wide groups plus a ragged tail."""
    out = []
    base = 0
    for w in (8, 8, 16):
        if base + w <= nchunk:
            out.append((base, w))
            base += w
    while base < nchunk:
        w = min(AW, nchunk - base)
        out.append((base, w))
        base += w
    return out


def _plan_units(cb):
    """Walk chunks in even-first block order; greedily pair same-bucket
    chunks that sit in the same DMA group (DoubleRow), singles otherwise.

    Returns (units, half_end_unit, nchunk) with units = [(bucket, c0, k)]."""
    nchunk = sum(cb)
    ends = {base + w - 1 for base, w in _chunk_groups(nchunk)}
    units = []
    c = 0
    for b in ORDER:
        rem = cb[b]
        while rem:
            if rem >= 2 and c not in ends and MMDT is F8:
                units.append((b, c, 2))
                c += 2
                rem -= 2
            else:
                units.append((b, c, 1))
                c += 1
                rem -= 1
    assert c == nchunk
    first8 = set(ORDER[:8])
    half_end_unit = max(i for i, u in enumerate(units) if u[0] in first8)
    return units, half_end_unit, nchunk


def build_nc(cb):
    """SPMD Bass program; cb[b] = chunks assigned to graph block b."""
    nchunk = sum(cb)
    assert nchunk % A == 0
    groups32 = _chunk_groups(nchunk)
    ngrp = len(groups32)
    units, half_end_unit, nck = _plan_units(cb)
    assert nck == nchunk
    first_unit = {}
    last_unit = {}
    for i, (b, _, _) in enumerate(units):
        first_unit.setdefault(b, i)
        last_unit[b] = i
    # units grouped by DMA group
    grp_of = {}
    for gi, (base, w) in enumerate(groups32):
        for c in range(base, base + w):
            grp_of[c] = gi
    sup_units = [[] for _ in range(ngrp)]
    for i, (b, c0, k) in enumerate(units):
        sup_units[grp_of[c0]].append((i, b, c0, k))
    slot = {b: ORDER.index(b) % 8 for b in range(NBUK)}

    nc = bacc.Bacc(
        "TRN2",
        target_bir_lowering=False,
        debug=False,
        num_devices=NCORES,
    )
    groups = [list(range(NCORES))]
    EQ = mybir.AluOpType.is_equal
    ADD = mybir.AluOpType.add
    MUL = mybir.AluOpType.mult
    SUB = mybir.AluOpType.subtract
    BYP = mybir.AluOpType.bypass
    AF = mybir.ActivationFunctionType

    # ---- I/O ----
    xp_d = nc.dram_tensor("xp", [nchunk * P * D], MMDT, kind="ExternalInput").ap()
    xo_d = nc.dram_tensor("xo", [nchunk * P * D], MMDT, kind="ExternalInput").ap()
    idx_d = nc.dram_tensor("idx", [P, 2, nchunk], BF16, kind="ExternalInput").ap()
    lnc_d = nc.dram_tensor("lnc", [P, 2, G], F32, kind="ExternalInput").ap()
    loss_out = nc.dram_tensor("loss", [1, 1], F32, kind="ExternalOutput").ap()

    # ---- internal DRAM ----
    # per-parity stages in natural block order: stage[t][h] = block 2h+t
    p_stage = nc.dram_tensor("p_stage", [2, 8, P, D], F8).ap()
    s_stage = nc.dram_tensor("s_stage", [2, 8, P, D], F8).ap()
    # AllToAll outputs: recv[t][q] = core q's partial of block 2r+t (r = me)
    p_recv = nc.dram_tensor("p_recv", [2, 8, P, D], F8).ap()
    s_recv = nc.dram_tensor("s_recv", [2, 8, P, D], F8).ap()
    # my normalized+transposed pos slice, and the AllGathered full table
    pnT_mine_d = nc.dram_tensor("pnT_mine", [P, 2, 2 * P], F8).ap()
    pnT_all = nc.dram_tensor(
        "pnT_all", [NCORES, P, 2, 2 * P], F8, addr_space="Shared"
    ).ap()

    with tile.TileContext(nc) as tc:
        with (
            tc.tile_pool(name="const", bufs=1) as cpool,
            tc.tile_pool(name="big", bufs=1) as big,
        ):
            # ---- constants / one-shot loads ----
            iota_i = cpool.tile([P, P], mybir.dt.int32, tag="iota_i")
            nc.gpsimd.iota(iota_i[:], pattern=[[1, P]], base=0, channel_multiplier=0)
            iota_bf = cpool.tile([P, P], BF16, tag="iota_bf")
            nc.vector.tensor_copy(iota_bf[:], iota_i[:])
            iota_1 = iota_bf[:].rearrange("p (o x) -> p o x", o=1)
            eps_col = cpool.tile([P, 1], F32, tag="eps_col")
            nc.vector.memset(eps_col[:], 1e-16)
            from concourse.masks import make_identity

            ident_f = cpool.tile([P, P], F32, tag="ident_f")
            make_identity(nc, ident_f[:])
            ident = cpool.tile([P, P], BF16, tag="ident")
            nc.vector.tensor_copy(ident[:], ident_f[:])

            it_sb = cpool.tile([P, 2, nchunk], BF16, tag="it")
            nc.sync.dma_start(out=it_sb[:], in_=idx_d)
            lnc_sb = big.tile([P, 2, G], F32, tag="lnc")
            nc.scalar.dma_start(out=lnc_sb[:], in_=lnc_d)

            # ============ segment-sum phase ============
            def seg_phase(x_d, t_row, stage_ap, psum_bufs, tag, on_half, on_full):
                """fp8 DoubleRow bucketed matmuls -> fp8 stage halves.

                stage_ap(par) -> [8, P, D] dram AP for that parity."""
                with (
                    tc.tile_pool(name=f"ps_{tag}", bufs=psum_bufs, space="PSUM") as pseg,
                    tc.tile_pool(name=f"st_{tag}", bufs=3) as stream,
                    tc.tile_pool(name=f"oh_{tag}", bufs=3) as ohp,
                    tc.tile_pool(name=f"sb_{tag}", bufs=2) as segsb,
                ):
                    acc = pseg.tile([P, 8, D], F32, tag="acc")
                    for gi, (base, w) in enumerate(groups32):
                        xt = stream.tile([P, w, D], MMDT, tag=f"xt{w}")
                        nc.sync.dma_start(
                            out=xt[:],
                            in_=x_d[base * P * D : (base + w) * P * D].rearrange(
                                "(p a d) -> p a d", p=P, a=w, d=D
                            ),
                        )
                        ohm = ohp.tile([P, w, P], MMDT, tag=f"ohm{w}")
                        it_b = (
                            it_sb[:, t_row, base : base + w]
                            .rearrange("p (a o) -> p a o", o=1)
                            .broadcast_to([P, w, P])
                        )
                        nc.vector.tensor_tensor(
                            out=ohm[:], in0=iota_1.broadcast_to([P, w, P]),
                            in1=it_b, op=EQ,
                        )
                        for i, b, c0, k in sup_units[gi]:
                            a = c0 - base
                            if k == 2:
                                nc.tensor.matmul(
                                    out=acc[:, slot[b], :],
                                    lhsT=ohm[:, a : a + 2, :],
                                    rhs=xt[:, a : a + 2, :],
                                    start=(i == first_unit[b]),
                                    stop=(i == last_unit[b]),
                                    perf_mode=mybir.MatmulPerfMode.DoubleRow,
                                )
                            else:
                                nc.tensor.matmul(
                                    out=acc[:, slot[b], :],
                                    lhsT=ohm[:, a, :],
                                    rhs=xt[:, a, :],
                                    start=(i == first_unit[b]),
                                    stop=(i == last_unit[b]),
                                )
                            if i == half_end_unit:
                                sbh = segsb.tile([P, 8, D], F8, tag="sbh")
                                nc.scalar.copy(sbh[:], acc[:])
                                nc.scalar.dma_start(
                                    out=stage_ap(0).rearrange("h p d -> p h d"),
                                    in_=sbh[:],
                                )
                                on_half()
                                acc = pseg.tile([P, 8, D], F32, tag="acc")
                    sbh = segsb.tile([P, 8, D], F8, tag="sbh")
                    nc.scalar.copy(sbh[:], acc[:])
                    nc.scalar.dma_start(
                        out=stage_ap(1).rearrange("h p d -> p h d"), in_=sbh[:]
                    )
                    on_full()

            def at_exchange(stage, recv, par):
                nc.gpsimd.collective_compute(
                    "AllToAll",
                    BYP,
                    replica_groups=groups,
                    ins=[stage[par]],
                    outs=[recv[par : par + 1]],
                )

            # ---- phase P (pos): per-parity AllToAll as halves land ----
            seg_phase(
                xp_d, 0, lambda par: p_stage[par], 2, "p",
                on_half=lambda: at_exchange(p_stage, p_recv, 0),
                on_full=lambda: at_exchange(p_stage, p_recv, 1),
            )

            # ---- pos exchange processing (gpsimd/scalar; hidden under O) ----
            # 8-way tree sum of my 2 blocks' partials, f32
            pr_sb = big.tile([P, 2, 8, D], F8, tag="pr")
            pt4 = big.tile([P, 2, 4, D], F32, tag="pt4")
            pt2 = big.tile([P, 2, 2, D], F32, tag="pt2")
            pl_f32 = big.tile([P, 2, D], F32, tag="pl")
            for par in range(2):
                nc.gpsimd.dma_start(
                    out=pr_sb[:, par],
                    in_=p_recv[par].rearrange("c p d -> p c d"),
                )
                nc.gpsimd.tensor_tensor(
                    out=pt4[:, par], in0=pr_sb[:, par, 0:4], in1=pr_sb[:, par, 4:8],
                    op=ADD,
                )
                nc.gpsimd.tensor_tensor(
                    out=pt2[:, par], in0=pt4[:, par, 0:2], in1=pt4[:, par, 2:4],
                    op=ADD,
                )
                nc.gpsimd.tensor_tensor(
                    out=pl_f32[:, par, :].rearrange("p (o d) -> p o d", o=1),
                    in0=pt2[:, par, 0:1], in1=pt2[:, par, 1:2],
                    op=ADD,
                )

            # inverse norms for my raw pos blocks (numerator + normalize);
            # scalar-engine Square+accum / Rsqrt keep the DVE queue clear
            # for phase-O one-hots
            sqp = big.tile([P, 2, D], F32, tag="sqp")
            n2p = big.tile([P, 2], F32, tag="n2p")
            for par in range(2):
                nc.scalar.activation(
                    out=sqp[:, par, :],
                    in_=pl_f32[:, par, :],
                    func=AF.Square,
                    accum_out=n2p[:, par : par + 1],
                )
            # rsqrt on the scalar engine via exp(-0.5*ln(x)) (Rsqrt is
            # blocked in bass; Ln/Exp tables are needed by the tail anyway)
            lnp = big.tile([P, 2], F32, tag="lnp")
            nc.scalar.activation(out=lnp[:], in_=n2p[:], func=AF.Ln, bias=eps_col[:])
            invp = big.tile([P, 2], F32, tag="invp")
            nc.scalar.activation(out=invp[:], in_=lnp[:], func=AF.Exp, scale=-0.5)

            # normalize my 2 pos blocks -> bf16, PE-transpose to [d, g] slice
            phn = big.tile([P, 2, D], BF16, tag="phn")
            for par in range(2):
                nc.gpsimd.tensor_scalar(
                    out=phn[:, par, :],
                    in0=pl_f32[:, par, :],
                    scalar1=invp[:, par : par + 1],
                    scalar2=None,
                    op0=MUL,
                )
            pnT_mine = big.tile([P, 2, 2 * P], F8, tag="pnT_mine")
            with tc.tile_pool(name="ps_trp", bufs=2, space="PSUM") as ptrp:
                for par in range(2):
                    for db in range(2):
                        tps = ptrp.tile([P, P], BF16, tag="trp")
                        nc.tensor.transpose(
                            out=tps[:],
                            in_=phn[:, par, db * P : (db + 1) * P],
                            identity=ident[:],
                        )
                        nc.scalar.copy(pnT_mine[:, db, par * P : (par + 1) * P], tps[:])
            nc.gpsimd.dma_start(out=pnT_mine_d, in_=pnT_mine[:])
            # share my pn_T slice with everyone (shared-output AllGather)
            nc.gpsimd.collective_compute(
                "AllGather",
                BYP,
                replica_groups=groups,
                ins=[pnT_mine_d],
                outs=[pnT_all[:]],
            )
            # full column table: col-block k (128 cols) = graph block k
            pn_T = big.tile([P, 2, G], F8, tag="pn_T")
            nc.gpsimd.dma_start(
                out=pn_T[:].rearrange("p h (c x) -> p h c x", c=NCORES),
                in_=pnT_all.rearrange("c p h x -> p h c x"),
            )

            # ---- phase O (origin/self): per-parity AllToAll ----
            seg_phase(
                xo_d, 1, lambda par: s_stage[par], 1, "o",
                on_half=lambda: at_exchange(s_stage, s_recv, 0),
                on_full=lambda: at_exchange(s_stage, s_recv, 1),
            )

            # ================= tail =================
            # self: 8-way tree sum (vector; DVE is free now)
            sr_sb = big.tile([P, 2, 8, D], F8, tag="sr")
            st4 = big.tile([P, 2, 4, D], F32, tag="st4")
            st2 = big.tile([P, 2, 2, D], F32, tag="st2")
            sl_f32 = big.tile([P, 2, D], F32, tag="sl")
            sqs = big.tile([P, 2, D], F32, tag="sqs")
            n2s = big.tile([P, 2], F32, tag="n2s")
            lns = big.tile([P, 2], F32, tag="lns")
            invs = big.tile([P, 2], F32, tag="invs")
            shat = big.tile([P, 2, D], BF16, tag="shat")
            sn_T = big.tile([P, 2, 2 * P], F8, tag="sn_T")
            with tc.tile_pool(name="ps_tr", bufs=2, space="PSUM") as ptr:
                for par in range(2):
                    # par 0 lands mid-phase-O: process on gpsimd (idle);
                    # par 1 arrives after phase O: DVE is free and faster.
                    eng = nc.gpsimd if par == 0 else nc.vector
                    dma_eng = nc.gpsimd if par == 0 else nc.sync
                    dma_eng.dma_start(
                        out=sr_sb[:, par],
                        in_=s_recv[par].rearrange("c p d -> p c d"),
                    )
                    eng.tensor_tensor(
                        out=st4[:, par], in0=sr_sb[:, par, 0:4],
                        in1=sr_sb[:, par, 4:8], op=ADD,
                    )
                    eng.tensor_tensor(
                        out=st2[:, par], in0=st4[:, par, 0:2], in1=st4[:, par, 2:4],
                        op=ADD,
                    )
                    eng.tensor_tensor(
                        out=sl_f32[:, par, :].rearrange("p (o d) -> p o d", o=1),
                        in0=st2[:, par, 0:1], in1=st2[:, par, 1:2],
                        op=ADD,
                    )
                    nc.scalar.activation(
                        out=sqs[:, par, :],
                        in_=sl_f32[:, par, :],
                        func=AF.Square,
                        accum_out=n2s[:, par : par + 1],
                    )
                    nc.scalar.activation(
                        out=lns[:, par : par + 1],
                        in_=n2s[:, par : par + 1],
                        func=AF.Ln,
                        bias=eps_col[:],
                    )
                    nc.scalar.activation(
                        out=invs[:, par : par + 1],
                        in_=lns[:, par : par + 1],
                        func=AF.Exp,
                        scale=-0.5,
                    )
                    eng.tensor_scalar(
                        out=shat[:, par, :],
                        in0=sl_f32[:, par, :],
                        scalar1=invs[:, par : par + 1],
                        scalar2=None,
                        op0=MUL,
                    )
                    for db in range(2):
                        tps = ptr.tile([P, P], BF16, tag="tr")
                        nc.tensor.transpose(
                            out=tps[:],
                            in_=shat[:, par, db * P : (db + 1) * P],
                            identity=ident[:],
                        )
                        nc.scalar.copy(sn_T[:, db, par * P : (par + 1) * P], tps[:])

            # numerator: sim0[p] = sum_par <s_raw, p_raw> * invs * invp
            rd = big.tile([P, 2, D], F32, tag="rd")
            nc.vector.tensor_tensor(out=rd[:], in0=sl_f32[:], in1=pl_f32[:], op=MUL)
            rd2 = big.tile([P, 2], F32, tag="rd2")
            nc.vector.tensor_reduce(
                out=rd2[:], in_=rd[:], axis=mybir.AxisListType.X, op=ADD
            )
            s0a = big.tile([P, 2], F32, tag="s0a")
            nc.vector.tensor_tensor(out=s0a[:], in0=rd2[:], in1=invs[:], op=MUL)
            s0b = big.tile([P, 2], F32, tag="s0b")
            nc.vector.tensor_tensor(out=s0b[:], in0=s0a[:], in1=invp[:], op=MUL)
            sim0 = big.tile([P, 1], F32, tag="sim0")
            nc.vector.tensor_reduce(
                out=sim0[:], in_=s0b[:], axis=mybir.AxisListType.X, op=ADD
            )

            # ---- Gram + loss: per (row-block lo, column-parity) ----
            denp = big.tile([P, 2], F32, tag="denp")
            with (
                tc.tile_pool(name="ps_gram", bufs=2, space="PSUM") as pgram,
                tc.tile_pool(name="gl", bufs=2) as gl,
            ):
                for lo in range(2):
                    pg = pgram.tile([P, 4, 512], F32, tag="pg")
                    for q in range(4):
                        nc.tensor.matmul(
                            out=pg[:, q, :],
                            lhsT=sn_T[:, :, lo * P : (lo + 1) * P],
                            rhs=pn_T[:, :, q * 512 : (q + 1) * 512],
                            start=True,
                            stop=True,
                            perf_mode=mybir.MatmulPerfMode.DoubleRow,
                        )
                    simln = gl.tile([P, G], F32, tag="simln")
                    nc.vector.tensor_tensor(
                        out=simln[:],
                        in0=pg[:].rearrange("p a b -> p (a b)"),
                        in1=lnc_sb[:, lo, :],
                        op=ADD,
                    )
                    ed = gl.tile([P, G], BF16, tag="ed")
                    nc.scalar.activation(
                        out=ed[:],
                        in_=simln[:],
                        func=AF.Exp,
                        accum_out=denp[:, lo : lo + 1],
                    )

            lden2 = big.tile([P, 2], F32, tag="lden2")
            nc.scalar.activation(out=lden2[:], in_=denp[:], func=AF.Ln)
            t0 = big.tile([P, 1], F32, tag="t0")
            nc.vector.tensor_reduce(
                out=t0[:], in_=lden2[:], axis=mybir.AxisListType.X, op=ADD
            )
            t1 = big.tile([P, 1], F32, tag="t1")
            nc.vector.tensor_tensor(out=t1[:], in0=t0[:], in1=sim0[:], op=SUB)
            ones_col = big.tile([P, 1], F32, tag="ones_col")
            nc.vector.memset(ones_col[:], 1.0)
            with tc.tile_pool(name="ps_ls", bufs=1, space="PSUM") as pls:
                lps = pls.tile([1, 1], F32, tag="lps")
                nc.tensor.matmul(
                    out=lps[:], lhsT=t1[:], rhs=ones_col[:], start=True, stop=True
                )
                lsum1 = big.tile([1, 1], F32, tag="lsum1")
                nc.scalar.copy(lsum1[:], lps[:])
            nc.sync.dma_start(out=loss_out[:], in_=lsum1[:])
    nc.compile()
    return nc


def _chunk_plan(idx_list):
    """cb[b] = chunk count covering max bucket occupancy over all
    (core, table) shards; total padded to a multiple of A."""
    maxc = np.zeros(NBUK, np.int64)
    for gids in idx_list:
        cnt = np.bincount((gids >> 7).astype(np.int64), minlength=NBUK)
        maxc = np.maximum(maxc, cnt)
    cb = [max(1, int(np.ceil(c / P))) for c in maxc]
    i = 0
    while sum(cb) % A != 0:
        cb[i % NBUK] += 1
        i += 1
    return cb


def _pack_shard(x, gids, cb, np_mm):
    """Order a core's nodes bucket-major (even blocks first) into the
    padded chunk layout.

    Returns (x_packed [nsup, P, A, D] np_mm, idx_rel [P, nchunk])."""
    nchunk = sum(cb)
    key = (gids >> 7).astype(np.int64)
    counts = np.bincount(key, minlength=NBUK)
    off = {}
    c = 0
    for b in ORDER:
        off[b] = c * P
        c += cb[b]
    pos_in_order = np.asarray([ORDER.index(b) for b in range(NBUK)], np.int64)
    order = np.argsort(pos_in_order[key], kind="stable")
    dst = np.concatenate([off[b] + np.arange(counts[b]) for b in ORDER])
    xpad = np.zeros((nchunk * P, D), np.float32)
    ipad = np.full((nchunk * P,), -1.0, np.float32)
    xpad[dst] = x[order]
    ipad[dst] = (gids[order] & 127).astype(np.float32)
    blocks = []
    for base, w in _chunk_groups(nchunk):
        blk = xpad[base * P : (base + w) * P].reshape(w, P, D).transpose(1, 0, 2)
        blocks.append(blk.reshape(-1))
    x_packed = np.concatenate(blocks).astype(np_mm)
    idx_rel = np.ascontiguousarray(ipad.reshape(nchunk, P).T)
    return x_packed, idx_rel


def _prep_inputs(logits_origin, logits_pos, ori_idx, pos_idx, neg_idx):
    import ml_dtypes  # noqa: F401

    np_mm = np.dtype(mybir.dt.np(MMDT))
    np_bf = np.dtype(mybir.dt.np(BF16))
    xo = np.ascontiguousarray(np.asarray(logits_origin, dtype=np.float32))
    xp = np.ascontiguousarray(np.asarray(logits_pos, dtype=np.float32))
    oi = np.asarray(ori_idx).astype(np.int64)
    pi = np.asarray(pos_idx).astype(np.int64)
    neg = np.asarray(neg_idx)
    n = xo.shape[0]
    assert xo.shape == (n, D) and xp.shape == (n, D)
    assert neg.shape == (G, S)

    nloc = (n + NCORES - 1) // NCORES
    shards = []
    for r in range(NCORES):
        lo = r * nloc
        hi = min(n, lo + nloc)
        shards.append((xo[lo:hi], oi[lo:hi], xp[lo:hi], pi[lo:hi]))
    cb = _chunk_plan([s[1] for s in shards] + [s[3] for s in shards])

    cnt = np.zeros((G, G), dtype=np.float64)
    rows = np.repeat(np.arange(G), S)
    np.add.at(cnt, (rows, neg.ravel().astype(np.int64)), 1.0)
    with np.errstate(divide="ignore"):
        lncnt = np.where(cnt > 0, np.log(cnt), -30000.0).astype(np.float32)

    in_maps = []
    for r in range(NCORES):
        xo_r, oi_r, xp_r, pi_r = shards[r]
        xp_pk, ip_rel = _pack_shard(xp_r, pi_r, cb, np_mm)
        xo_pk, io_rel = _pack_shard(xo_r, oi_r, cb, np_mm)
        idx_pk = np.stack([ip_rel, io_rel], axis=1).astype(np_bf)  # [P, 2, nchunk]
        # local graphs = blocks {2r, 2r+1} = [256r, 256r+256)
        lnc_r = np.stack(
            [lncnt[r * GLOC + lo * P : r * GLOC + (lo + 1) * P] for lo in range(2)],
            axis=1,
        ).astype(np.float32)  # [P, 2, G]
        in_maps.append(
            {
                "xp": xp_pk,
                "xo": xo_pk,
                "idx": np.ascontiguousarray(idx_pk),
                "lnc": np.ascontiguousarray(lnc_r),
            }
        )
    return cb, in_maps


def kernel(
    logits_origin,
    logits_pos,
    ori_idx,
    pos_idx,
    neg_idx,
    _trace=False,
    _tmpdir=None,
):
    cb, in_maps = _prep_inputs(logits_origin, logits_pos, ori_idx, pos_idx, neg_idx)
    if _trace:
        _ensure_ntff_hook()
    nc = build_nc(cb)
    res = run_bass_kernel_spmd(
        nc,
        in_maps,
        core_ids=list(range(NCORES)),
        trace=_trace,
        tmpdir=_tmpdir,
    )
    kernel._last_results = res
    total = sum(float(res.results[r]["loss"][0, 0]) for r in range(NCORES))
    return np.asarray(np.float32(total / G))


kernel._last_results = None


if __name__ == "__main__":
    rng = np.random.default_rng(0)
    n = 4096
    inputs = {
        "logits_origin": rng.standard_normal((n, D), dtype=np.float32),
        "logits_pos": rng.standard_normal((n, D), dtype=np.float32),
        "ori_idx": rng.integers(0, G, n, dtype=np.int32),
        "pos_idx": rng.integers(0, G, n, dtype=np.int32),
        "neg_idx": rng.integers(0, G, (G, S), dtype=np.int32),
    }

    def np_ref(logits_origin, logits_pos, ori_idx, pos_idx, neg_idx):
        x = logits_origin.astype(np.float64)
        y = logits_pos.astype(np.float64)
        self_l = np.zeros((G, D))
        pos_l = np.zeros((G, D))
        np.add.at(self_l, ori_idx, x)
        np.add.at(pos_l, pos_idx, y)
        eps = 1e-8
        na = np.maximum(np.linalg.norm(self_l, axis=1), eps)
        nb = np.maximum(np.linalg.norm(pos_l, axis=1), eps)
        sh = self_l / na[:, None]
        ph = pos_l / nb[:, None]
        gram = sh @ ph.T
        sim0 = np.einsum("gd,gd->g", sh, ph)
        e = np.exp(gram)
        den = np.array([e[g, neg_idx[g]].sum() for g in range(G)])
        res = np.log(den) - sim0
        return res.mean()

    expected = np_ref(**inputs)
    actual = kernel(**inputs)
    err = abs(actual - expected) / max(abs(expected), 1e-12)
    print(f"expected={expected:.6f} actual={float(actual):.6f} relerr={err:.3e}")

# revision 18
# speedup vs baseline: 1.0673x; 1.0673x over previous
"""Trainium2 Bass kernel for graph-contrastive loss (nn_PrePrompt_75496935129282).

Computation (reference):
    self = segment_sum(logits_origin, ori_idx, G)       # [G, D]
    pos  = segment_sum(logits_pos,  pos_idx, G)         # [G, D]
    sim[g, k]  = cos(self[g], pos[k])   (eps-guarded norms)
    res[g] = log(sum_s exp(sim[g, neg_idx[g, s]])) - sim[g, g]
    out = mean(res)

Device strategy (8 NeuronCores, SPMD):
  - Nodes sharded 8 ways. Host orders each core's nodes by graph block
    (gid >> 7, 16 blocks of 128) with data-driven per-block chunk
    counts: every 128-node chunk targets one block, so the one-hot
    matmul is [128, 128] per chunk. Blocks are processed EVENS FIRST
    so PSUM/stage halves split by block parity.
  - Streams are fp8e4m3; chunk PAIRS go through one DoubleRow matmul
    (256 nodes per PE pass at 0.5 cycles/row); odd tails use a single
    fp8 matmul. One-hots for 16 chunks are generated with a single
    broadcast is_equal against a block-relative bf16 iota.
  - Exchange (no ring collectives): per-parity AllToAll redistributes
    raw per-core partials (core r receives all 8 cores' partials of
    blocks {2r, 2r+1}); the 8-way sum happens locally (tree adds, f32)
    on gpsimd (pos, hidden under the origin phase) / vector (self).
    Each core normalizes + PE-transposes only its OWN 2 pos blocks and
    a single shared-output AllGather of the 64KB pn_T slice rebuilds
    the full [128d, 2, 2048] column table on every core.
  - Tail: cosine Gram of the core's 256 self rows vs all 2048 columns,
    denominator = exp-accumulate of (sim + ln(count)) with a
    host-precomputed f32 count table, numerator from the raw f32 local
    sums scaled by inverse norms. Per-core losses summed on host.
"""

import os
import sys

sys.path.insert(0, "/opt/trn_rl_repo")

import numpy as np

import concourse.bacc as bacc
import concourse.bass as bass  # noqa: F401
import concourse.mybir as mybir
import concourse.tile as tile
from concourse import bass_isa
from concourse.bass_utils import run_bass_kernel_spmd


def _ensure_ntff_hook():
    """The agent image's antenv lacks axon_hooks; inject it and register
    the ctypes NTFF profiling hook so trace=True works under axon."""
    import types

    import antenv

    if hasattr(antenv, "axon_hooks"):
        return
    mod = types.ModuleType("antenv.axon_hooks")
    mod._hook = None

    def set_axon_ntff_profile_hook(h):
        mod._hook = h

    def get_axon_ntff_profile_hook():
        return mod._hook

    mod.set_axon_ntff_profile_hook = set_axon_ntff_profile_hook
    mod.get_axon_ntff_profile_hook = get_axon_ntff_profile_hook
    sys.modules["antenv.axon_hooks"] = mod
    antenv.axon_hooks = mod
    try:
        from trn_agent_boot.trn_boot import _ntff_profile_via_ctypes

        mod._hook = _ntff_profile_via_ctypes("/opt/axon/libaxon_pjrt.so")
    except Exception as e:  # pragma: no cover
        print(f"ntff hook registration failed: {e}")


F32 = mybir.dt.float32
BF16 = mybir.dt.bfloat16
F8 = mybir.dt.float8e4

G = 2048
S = 127
D = 256
NCORES = 8
P = 128
A = 16  # chunk-count quantum (padding granularity)
AW = 32  # chunks per DMA group (packet size = AW*D fp8 per partition)
NBUK = 16  # graph blocks of 128
GLOC = G // NCORES  # 256

# even blocks first, then odd: PSUM generation k holds blocks ORDER[8k:8k+8]
ORDER = list(range(0, NBUK, 2)) + list(range(1, NBUK, 2))

_MM_RAW = os.environ.get("KERNEL_MM_DT", "f8")
MMDT = BF16 if _MM_RAW == "bf16" else F8


def _chunk_groups(nchunk):
    """DMA groups: small warm-up groups (fast pipeline start), then
    AW-wide groups plus a ragged tail."""
    out = []
    base = 0
    for w in (8, 8, 16):
        if base + w <= nchunk:
            out.append((base, w))
            base += w
    while base < nchunk:
        w = min(AW, nchunk - base)
        out.append((base, w))
        base += w
    return out


def _plan_units(cb):
    """Walk chunks in even-first block order; greedily pair same-bucket
    chunks that sit in the same DMA group (DoubleRow), singles otherwise.

    Returns (units, half_end_unit, nchunk) with units = [(bucket, c0, k)]."""
    nchunk = sum(cb)
    ends = {base + w - 1 for base, w in _chunk_groups(nchunk)}
    units = []
    c = 0
    for b in ORDER:
        rem = cb[b]
        while rem:
            if rem >= 2 and c not in ends and MMDT is F8:
                units.append((b, c, 2))
                c += 2
                rem -= 2
            else:
                units.append((b, c, 1))
                c += 1
                rem -= 1
    assert c == nchunk
    first8 = set(ORDER[:8])
    half_end_unit = max(i for i, u in enumerate(units) if u[0] in first8)
    return units, half_end_unit, nchunk


def build_nc(cb):
    """SPMD Bass program; cb[b] = chunks assigned to graph block b."""
    nchunk = sum(cb)
    assert nchunk % A == 0
    groups32 = _chunk_groups(nchunk)
    ngrp = len(groups32)
    units, half_end_unit, nck = _plan_units(cb)
    assert nck == nchunk
    first_unit = {}
    last_unit = {}
    for i, (b, _, _) in enumerate(units):
        first_unit.setdefault(b, i)
        last_unit[b] = i
    # units grouped by DMA group
    grp_of = {}
    for gi, (base, w) in enumerate(groups32):
        for c in range(base, base + w):
            grp_of[c] = gi
    sup_units = [[] for _ in range(ngrp)]
    for i, (b, c0, k) in enumerate(units):
        sup_units[grp_of[c0]].append((i, b, c0, k))
    slot = {b: ORDER.index(b) % 8 for b in range(NBUK)}

    nc = bacc.Bacc(
        "TRN2",
        target_bir_lowering=False,
        debug=False,
        num_devices=NCORES,
    )
    groups = [list(range(NCORES))]
    EQ = mybir.AluOpType.is_equal
    ADD = mybir.AluOpType.add
    MUL = mybir.AluOpType.mult
    SUB = mybir.AluOpType.subtract
    BYP = mybir.AluOpType.bypass
    AF = mybir.ActivationFunctionType

    # ---- I/O ----
    xp_d = nc.dram_tensor("xp", [nchunk * P * D], MMDT, kind="ExternalInput").ap()
    xo_d = nc.dram_tensor("xo", [nchunk * P * D], MMDT, kind="ExternalInput").ap()
    idx_d = nc.dram_tensor("idx", [P, 2, nchunk], BF16, kind="ExternalInput").ap()
    lnc_d = nc.dram_tensor("lnc", [P, 2, G], F32, kind="ExternalInput").ap()
    loss_out = nc.dram_tensor("loss", [1, 1], F32, kind="ExternalOutput").ap()

    # ---- internal DRAM ----
    # per-parity stages in natural block order: stage[t][h] = block 2h+t
    p_stage = nc.dram_tensor("p_stage", [2, 8, P, D], F8).ap()
    s_stage = nc.dram_tensor("s_stage", [2, 8, P, D], F8).ap()
    # AllToAll outputs: recv[t][q] = core q's partial of block 2r+t (r = me)
    p_recv = nc.dram_tensor("p_recv", [2, 8, P, D], F8).ap()
    s_recv = nc.dram_tensor("s_recv", [2, 8, P, D], F8).ap()
    # my normalized+transposed pos slice, and the AllGathered full table
    pnT_mine_d = nc.dram_tensor("pnT_mine", [P, 2, 2 * P], F8).ap()
    pnT_all = nc.dram_tensor(
        "pnT_all", [NCORES, P, 2, 2 * P], F8, addr_space="Shared"
    ).ap()
    # warm-up collectives: the first collective after the kernel barrier
    # pays a ~47us channel-setup cost; burn it at t=0 under phase P
    warm_in = nc.dram_tensor("warm_in", [NCORES, 16], F8).ap()
    warm_at = nc.dram_tensor("warm_at", [NCORES, 16], F8).ap()
    warm_ag = nc.dram_tensor(
        "warm_ag", [NCORES, 16], F8, addr_space="Shared"
    ).ap()

    with tile.TileContext(nc) as tc:
        with (
            tc.tile_pool(name="const", bufs=1) as cpool,
            tc.tile_pool(name="big", bufs=1) as big,
        ):
            # warm up the collective channels immediately (no data deps)
            nc.gpsimd.collective_compute(
                "AllToAll", BYP, replica_groups=groups,
                ins=[warm_in], outs=[warm_at],
            )
            nc.gpsimd.collective_compute(
                "AllGather", BYP, replica_groups=groups,
                ins=[warm_in[0]], outs=[warm_ag],
            )

            # ---- constants / one-shot loads ----
            iota_i = cpool.tile([P, P], mybir.dt.int32, tag="iota_i")
            nc.gpsimd.iota(iota_i[:], pattern=[[1, P]], base=0, channel_multiplier=0)
            iota_bf = cpool.tile([P, P], BF16, tag="iota_bf")
            nc.vector.tensor_copy(iota_bf[:], iota_i[:])
            iota_1 = iota_bf[:].rearrange("p (o x) -> p o x", o=1)
            eps_col = cpool.tile([P, 1], F32, tag="eps_col")
            nc.vector.memset(eps_col[:], 1e-16)
            from concourse.masks import make_identity

            ident_f = cpool.tile([P, P], F32, tag="ident_f")
            make_identity(nc, ident_f[:])
            ident = cpool.tile([P, P], BF16, tag="ident")
            nc.vector.tensor_copy(ident[:], ident_f[:])

            it_sb = cpool.tile([P, 2, nchunk], BF16, tag="it")
            nc.sync.dma_start(out=it_sb[:], in_=idx_d)
            lnc_sb = big.tile([P, 2, G], F32, tag="lnc")
            nc.scalar.dma_start(out=lnc_sb[:], in_=lnc_d)

            # ============ segment-sum phase ============
            def seg_phase(x_d, t_row, stage_ap, psum_bufs, tag, on_half, on_full):
                """fp8 DoubleRow bucketed matmuls -> fp8 stage halves.

                stage_ap(par) -> [8, P, D] dram AP for that parity."""
                with (
                    tc.tile_pool(name=f"ps_{tag}", bufs=psum_bufs, space="PSUM") as pseg,
                    tc.tile_pool(name=f"st_{tag}", bufs=3) as stream,
                    tc.tile_pool(name=f"oh_{tag}", bufs=3) as ohp,
                    tc.tile_pool(name=f"sb_{tag}", bufs=2) as segsb,
                ):
                    acc = pseg.tile([P, 8, D], F32, tag="acc")
                    for gi, (base, w) in enumerate(groups32):
                        xt = stream.tile([P, w, D], MMDT, tag=f"xt{w}")
                        nc.sync.dma_start(
                            out=xt[:],
                            in_=x_d[base * P * D : (base + w) * P * D].rearrange(
                                "(p a d) -> p a d", p=P, a=w, d=D
                            ),
                        )
                        ohm = ohp.tile([P, w, P], MMDT, tag=f"ohm{w}")
                        it_b = (
                            it_sb[:, t_row, base : base + w]
                            .rearrange("p (a o) -> p a o", o=1)
                            .broadcast_to([P, w, P])
                        )
                        nc.vector.tensor_tensor(
                            out=ohm[:], in0=iota_1.broadcast_to([P, w, P]),
                            in1=it_b, op=EQ,
                        )
                        for i, b, c0, k in sup_units[gi]:
                            a = c0 - base
                            if k == 2:
                                nc.tensor.matmul(
                                    out=acc[:, slot[b], :],
                                    lhsT=ohm[:, a : a + 2, :],
                                    rhs=xt[:, a : a + 2, :],
                                    start=(i == first_unit[b]),
                                    stop=(i == last_unit[b]),
                                    perf_mode=mybir.MatmulPerfMode.DoubleRow,
                                )
                            else:
                                nc.tensor.matmul(
                                    out=acc[:, slot[b], :],
                                    lhsT=ohm[:, a, :],
                                    rhs=xt[:, a, :],
                                    start=(i == first_unit[b]),
                                    stop=(i == last_unit[b]),
                                )
                            if i == half_end_unit:
                                sbh = segsb.tile([P, 8, D], F8, tag="sbh")
                                nc.scalar.copy(sbh[:], acc[:])
                                nc.scalar.dma_start(
                                    out=stage_ap(0).rearrange("h p d -> p h d"),
                                    in_=sbh[:],
                                )
                                on_half()
                                acc = pseg.tile([P, 8, D], F32, tag="acc")
                    sbh = segsb.tile([P, 8, D], F8, tag="sbh")
                    nc.scalar.copy(sbh[:], acc[:])
                    nc.scalar.dma_start(
                        out=stage_ap(1).rearrange("h p d -> p h d"), in_=sbh[:]
                    )
                    on_full()

            def at_exchange(stage, recv, par):
                nc.gpsimd.collective_compute(
                    "AllToAll",
                    BYP,
                    replica_groups=groups,
                    ins=[stage[par]],
                    outs=[recv[par : par + 1]],
                )

            # ---- phase P (pos): per-parity AllToAll as halves land ----
            seg_phase(
                xp_d, 0, lambda par: p_stage[par], 2, "p",
                on_half=lambda: at_exchange(p_stage, p_recv, 0),
                on_full=lambda: at_exchange(p_stage, p_recv, 1),
            )

            # ---- pos exchange processing (gpsimd/scalar; hidden under O) ----
            # 8-way tree sum of my 2 blocks' partials, f32
            pr_sb = big.tile([P, 2, 8, D], F8, tag="pr")
            pt4 = big.tile([P, 2, 4, D], F32, tag="pt4")
            pt2 = big.tile([P, 2, 2, D], F32, tag="pt2")
            pl_f32 = big.tile([P, 2, D], F32, tag="pl")
            for par in range(2):
                nc.gpsimd.dma_start(
                    out=pr_sb[:, par],
                    in_=p_recv[par].rearrange("c p d -> p c d"),
                )
                nc.gpsimd.tensor_tensor(
                    out=pt4[:, par], in0=pr_sb[:, par, 0:4], in1=pr_sb[:, par, 4:8],
                    op=ADD,
                )
                nc.gpsimd.tensor_tensor(
                    out=pt2[:, par], in0=pt4[:, par, 0:2], in1=pt4[:, par, 2:4],
                    op=ADD,
                )
                nc.gpsimd.tensor_tensor(
                    out=pl_f32[:, par, :].rearrange("p (o d) -> p o d", o=1),
                    in0=pt2[:, par, 0:1], in1=pt2[:, par, 1:2],
                    op=ADD,
                )

            # inverse norms for my raw pos blocks (numerator + normalize);
            # scalar-engine Square+accum / Rsqrt keep the DVE queue clear
            # for phase-O one-hots
            sqp = big.tile([P, 2, D], F32, tag="sqp")
            n2p = big.tile([P, 2], F32, tag="n2p")
            for par in range(2):
                nc.scalar.activation(
                    out=sqp[:, par, :],
                    in_=pl_f32[:, par, :],
                    func=AF.Square,
                    accum_out=n2p[:, par : par + 1],
                )
            # rsqrt on the scalar engine via exp(-0.5*ln(x)) (Rsqrt is
            # blocked in bass; Ln/Exp tables are needed by the tail anyway)
            lnp = big.tile([P, 2], F32, tag="lnp")
            nc.scalar.activation(out=lnp[:], in_=n2p[:], func=AF.Ln, bias=eps_col[:])
            invp = big.tile([P, 2], F32, tag="invp")
            nc.scalar.activation(out=invp[:], in_=lnp[:], func=AF.Exp, scale=-0.5)

            # normalize my 2 pos blocks -> bf16, PE-transpose to [d, g] slice
            phn = big.tile([P, 2, D], BF16, tag="phn")
            for par in range(2):
                nc.gpsimd.tensor_scalar(
                    out=phn[:, par, :],
                    in0=pl_f32[:, par, :],
                    scalar1=invp[:, par : par + 1],
                    scalar2=None,
                    op0=MUL,
                )
            pnT_mine = big.tile([P, 2, 2 * P], F8, tag="pnT_mine")
            # keep ps_trp open across phase O so its banks stay disjoint
            # from the phase-O accumulator (overlapping lifetimes)
            with tc.tile_pool(name="ps_trp", bufs=2, space="PSUM") as ptrp:
                for par in range(2):
                    for db in range(2):
                        tps = ptrp.tile([P, P], BF16, tag="trp")
                        nc.tensor.transpose(
                            out=tps[:],
                            in_=phn[:, par, db * P : (db + 1) * P],
                            identity=ident[:],
                        )
                        nc.scalar.copy(pnT_mine[:, db, par * P : (par + 1) * P], tps[:])
                nc.gpsimd.dma_start(out=pnT_mine_d, in_=pnT_mine[:])
                # share my pn_T slice with everyone (shared-output AllGather)
                nc.gpsimd.collective_compute(
                    "AllGather",
                    BYP,
                    replica_groups=groups,
                    ins=[pnT_mine_d],
                    outs=[pnT_all[:]],
                )
                # full column table: col-block k (128 cols) = graph block k
                pn_T = big.tile([P, 2, G], F8, tag="pn_T")
                nc.gpsimd.dma_start(
                    out=pn_T[:].rearrange("p h (c x) -> p h c x", c=NCORES),
                    in_=pnT_all.rearrange("c p h x -> p h c x"),
                )

                # ---- phase O (origin/self): per-parity AllToAll ----
                seg_phase(
                    xo_d, 1, lambda par: s_stage[par], 1, "o",
                    on_half=lambda: at_exchange(s_stage, s_recv, 0),
                    on_full=lambda: at_exchange(s_stage, s_recv, 1),
                )

            # ================= tail =================
            # self: 8-way tree sum (vector; DVE is free now)
            sr_sb = big.tile([P, 2, 8, D], F8, tag="sr")
            st4 = big.tile([P, 2, 4, D], F32, tag="st4")
            st2 = big.tile([P, 2, 2, D], F32, tag="st2")
            sl_f32 = big.tile([P, 2, D], F32, tag="sl")
            sqs = big.tile([P, 2, D], F32, tag="sqs")
            n2s = big.tile([P, 2], F32, tag="n2s")
            lns = big.tile([P, 2], F32, tag="lns")
            invs = big.tile([P, 2], F32, tag="invs")
            shat = big.tile([P, 2, D], BF16, tag="shat")
            sn_T = big.tile([P, 2, 2 * P], F8, tag="sn_T")
            with tc.tile_pool(name="ps_tr", bufs=2, space="PSUM") as ptr:
                for par in range(2):
                    # par 0 lands mid-phase-O: process on gpsimd (idle);
                    # par 1 arrives after phase O: DVE is free and faster.
                    eng = nc.gpsimd if par == 0 else nc.vector
                    dma_eng = nc.gpsimd if par == 0 else nc.sync
                    dma_eng.dma_start(
                        out=sr_sb[:, par],
                        in_=s_recv[par].rearrange("c p d -> p c d"),
                    )
                    eng.tensor_tensor(
                        out=st4[:, par], in0=sr_sb[:, par, 0:4],
                        in1=sr_sb[:, par, 4:8], op=ADD,
                    )
                    eng.tensor_tensor(
                        out=st2[:, par], in0=st4[:, par, 0:2], in1=st4[:, par, 2:4],
                        op=ADD,
                    )
                    eng.tensor_tensor(
                        out=sl_f32[:, par, :].rearrange("p (o d) -> p o d", o=1),
                        in0=st2[:, par, 0:1], in1=st2[:, par, 1:2],
                        op=ADD,
                    )
                    nc.scalar.activation(
                        out=sqs[:, par, :],
                        in_=sl_f32[:, par, :],
                        func=AF.Square,
                        accum_out=n2s[:, par : par + 1],
                    )
                    nc.scalar.activation(
                        out=lns[:, par : par + 1],
                        in_=n2s[:, par : par + 1],
                        func=AF.Ln,
                        bias=eps_col[:],
                    )
                    nc.scalar.activation(
                        out=invs[:, par : par + 1],
                        in_=lns[:, par : par + 1],
                        func=AF.Exp,
                        scale=-0.5,
                    )
                    eng.tensor_scalar(
                        out=shat[:, par, :],
                        in0=sl_f32[:, par, :],
                        scalar1=invs[:, par : par + 1],
                        scalar2=None,
                        op0=MUL,
                    )
                    for db in range(2):
                        tps = ptr.tile([P, P], BF16, tag="tr")
                        nc.tensor.transpose(
                            out=tps[:],
                            in_=shat[:, par, db * P : (db + 1) * P],
                            identity=ident[:],
                        )
                        nc.scalar.copy(sn_T[:, db, par * P : (par + 1) * P], tps[:])

            # numerator: sim0[p] = sum_par <s_raw, p_raw> * invs * invp
            rd = big.tile([P, 2, D], F32, tag="rd")
            nc.vector.tensor_tensor(out=rd[:], in0=sl_f32[:], in1=pl_f32[:], op=MUL)
            rd2 = big.tile([P, 2], F32, tag="rd2")
            nc.vector.tensor_reduce(
                out=rd2[:], in_=rd[:], axis=mybir.AxisListType.X, op=ADD
            )
            s0a = big.tile([P, 2], F32, tag="s0a")
            nc.vector.tensor_tensor(out=s0a[:], in0=rd2[:], in1=invs[:], op=MUL)
            s0b = big.tile([P, 2], F32, tag="s0b")
            nc.vector.tensor_tensor(out=s0b[:], in0=s0a[:], in1=invp[:], op=MUL)
            sim0 = big.tile([P, 1], F32, tag="sim0")
            nc.vector.tensor_reduce(
                out=sim0[:], in_=s0b[:], axis=mybir.AxisListType.X, op=ADD
            )

            # ---- Gram + loss: per (row-block lo, column-parity) ----
            denp = big.tile([P, 2], F32, tag="denp")
            with (
                tc.tile_pool(name="ps_gram", bufs=2, space="PSUM") as pgram,
                tc.tile_pool(name="gl", bufs=2) as gl,
            ):
                for lo in range(2):
                    pg = pgram.tile([P, 4, 512], F32, tag="pg")
                    for q in range(4):
                        nc.tensor.matmul(
                            out=pg[:, q, :],
                            lhsT=sn_T[:, :, lo * P : (lo + 1) * P],
                            rhs=pn_T[:, :, q * 512 : (q + 1) * 512],
                            start=True,
                            stop=True,
                            perf_mode=mybir.MatmulPerfMode.DoubleRow,
                        )
                    simln = gl.tile([P, G], F32, tag="simln")
                    nc.vector.tensor_tensor(
                        out=simln[:],
                        in0=pg[:].rearrange("p a b -> p (a b)"),
                        in1=lnc_sb[:, lo, :],
                        op=ADD,
                    )
                    ed = gl.tile([P, G], BF16, tag="ed")
                    nc.scalar.activation(
                        out=ed[:],
                        in_=simln[:],
                        func=AF.Exp,
                        accum_out=denp[:, lo : lo + 1],
                    )

            lden2 = big.tile([P, 2], F32, tag="lden2")
            nc.scalar.activation(out=lden2[:], in_=denp[:], func=AF.Ln)
            t0 = big.tile([P, 1], F32, tag="t0")
            nc.vector.tensor_reduce(
                out=t0[:], in_=lden2[:], axis=mybir.AxisListType.X, op=ADD
            )
            t1 = big.tile([P, 1], F32, tag="t1")
            nc.vector.tensor_tensor(out=t1[:], in0=t0[:], in1=sim0[:], op=SUB)
            ones_col = big.tile([P, 1], F32, tag="ones_col")
            nc.vector.memset(ones_col[:], 1.0)
            with tc.tile_pool(name="ps_ls", bufs=1, space="PSUM") as pls:
                lps = pls.tile([1, 1], F32, tag="lps")
                nc.tensor.matmul(
                    out=lps[:], lhsT=t1[:], rhs=ones_col[:], start=True, stop=True
                )
                lsum1 = big.tile([1, 1], F32, tag="lsum1")
                nc.scalar.copy(lsum1[:], lps[:])
            nc.sync.dma_start(out=loss_out[:], in_=lsum1[:])
    nc.compile()
    return nc


def _chunk_plan(idx_list):
    """cb[b] = chunk count covering max bucket occupancy over all
    (core, table) shards; total padded to a multiple of A."""
    maxc = np.zeros(NBUK, np.int64)
    for gids in idx_list:
        cnt = np.bincount((gids >> 7).astype(np.int64), minlength=NBUK)
        maxc = np.maximum(maxc, cnt)
    cb = [max(1, int(np.ceil(c / P))) for c in maxc]
    i = 0
    while sum(cb) % A != 0:
        cb[i % NBUK] += 1
        i += 1
    return cb


def _pack_shard(x, gids, cb, np_mm):
    """Order a core's nodes bucket-major (even blocks first) into the
    padded chunk layout.

    Returns (x_packed [nsup, P, A, D] np_mm, idx_rel [P, nchunk])."""
    nchunk = sum(cb)
    key = (gids >> 7).astype(np.int64)
    counts = np.bincount(key, minlength=NBUK)
    off = {}
    c = 0
    for b in ORDER:
        off[b] = c * P
        c += cb[b]
    pos_in_order = np.asarray([ORDER.index(b) for b in range(NBUK)], np.int64)
    order = np.argsort(pos_in_order[key], kind="stable")
    dst = np.concatenate([off[b] + np.arange(counts[b]) for b in ORDER])
    xpad = np.zeros((nchunk * P, D), np.float32)
    ipad = np.full((nchunk * P,), -1.0, np.float32)
    xpad[dst] = x[order]
    ipad[dst] = (gids[order] & 127).astype(np.float32)
    blocks = []
    for base, w in _chunk_groups(nchunk):
        blk = xpad[base * P : (base + w) * P].reshape(w, P, D).transpose(1, 0, 2)
        blocks.append(blk.reshape(-1))
    x_packed = np.concatenate(blocks).astype(np_mm)
    idx_rel = np.ascontiguousarray(ipad.reshape(nchunk, P).T)
    return x_packed, idx_rel


def _prep_inputs(logits_origin, logits_pos, ori_idx, pos_idx, neg_idx):
    import ml_dtypes  # noqa: F401

    np_mm = np.dtype(mybir.dt.np(MMDT))
    np_bf = np.dtype(mybir.dt.np(BF16))
    xo = np.ascontiguousarray(np.asarray(logits_origin, dtype=np.float32))
    xp = np.ascontiguousarray(np.asarray(logits_pos, dtype=np.float32))
    oi = np.asarray(ori_idx).astype(np.int64)
    pi = np.asarray(pos_idx).astype(np.int64)
    neg = np.asarray(neg_idx)
    n = xo.shape[0]
    assert xo.shape == (n, D) and xp.shape == (n, D)
    assert neg.shape == (G, S)

    nloc = (n + NCORES - 1) // NCORES
    shards = []
    for r in range(NCORES):
        lo = r * nloc
        hi = min(n, lo + nloc)
        shards.append((xo[lo:hi], oi[lo:hi], xp[lo:hi], pi[lo:hi]))
    cb = _chunk_plan([s[1] for s in shards] + [s[3] for s in shards])

    cnt = np.zeros((G, G), dtype=np.float64)
    rows = np.repeat(np.arange(G), S)
    np.add.at(cnt, (rows, neg.ravel().astype(np.int64)), 1.0)
    with np.errstate(divide="ignore"):
        lncnt = np.where(cnt > 0, np.log(cnt), -30000.0).astype(np.float32)

    in_maps = []
    for r in range(NCORES):
        xo_r, oi_r, xp_r, pi_r = shards[r]
        xp_pk, ip_rel = _pack_shard(xp_r, pi_r, cb, np_mm)
        xo_pk, io_rel = _pack_shard(xo_r, oi_r, cb, np_mm)
        idx_pk = np.stack([ip_rel, io_rel], axis=1).astype(np_bf)  # [P, 2, nchunk]
        # local graphs = blocks {2r, 2r+1} = [256r, 256r+256)
        lnc_r = np.stack(
            [lncnt[r * GLOC + lo * P : r * GLOC + (lo + 1) * P] for lo in range(2)],
            axis=1,
        ).astype(np.float32)  # [P, 2, G]
        in_maps.append(
            {
                "xp": xp_pk,
                "xo": xo_pk,
                "idx": np.ascontiguousarray(idx_pk),
                "lnc": np.ascontiguousarray(lnc_r),
            }
        )
    return cb, in_maps


def kernel(
    logits_origin,
    logits_pos,
    ori_idx,
    pos_idx,
    neg_idx,
    _trace=False,
    _tmpdir=None,
):
    cb, in_maps = _prep_inputs(logits_origin, logits_pos, ori_idx, pos_idx, neg_idx)
    if _trace:
        _ensure_ntff_hook()
    nc = build_nc(cb)
    res = run_bass_kernel_spmd(
        nc,
        in_maps,
        core_ids=list(range(NCORES)),
        trace=_trace,
        tmpdir=_tmpdir,
    )
    kernel._last_results = res
    total = sum(float(res.results[r]["loss"][0, 0]) for r in range(NCORES))
    return np.asarray(np.float32(total / G))


kernel._last_results = None


if __name__ == "__main__":
    rng = np.random.default_rng(0)
    n = 4096
    inputs = {
        "logits_origin": rng.standard_normal((n, D), dtype=np.float32),
        "logits_pos": rng.standard_normal((n, D), dtype=np.float32),
        "ori_idx": rng.integers(0, G, n, dtype=np.int32),
        "pos_idx": rng.integers(0, G, n, dtype=np.int32),
        "neg_idx": rng.integers(0, G, (G, S), dtype=np.int32),
    }

    def np_ref(logits_origin, logits_pos, ori_idx, pos_idx, neg_idx):
        x = logits_origin.astype(np.float64)
        y = logits_pos.astype(np.float64)
        self_l = np.zeros((G, D))
        pos_l = np.zeros((G, D))
        np.add.at(self_l, ori_idx, x)
        np.add.at(pos_l, pos_idx, y)
        eps = 1e-8
        na = np.maximum(np.linalg.norm(self_l, axis=1), eps)
        nb = np.maximum(np.linalg.norm(pos_l, axis=1), eps)
        sh = self_l / na[:, None]
        ph = pos_l / nb[:, None]
        gram = sh @ ph.T
        sim0 = np.einsum("gd,gd->g", sh, ph)
        e = np.exp(gram)
        den = np.array([e[g, neg_idx[g]].sum() for g in range(G)])
        res = np.log(den) - sim0
        return res.mean()

    expected = np_ref(**inputs)
    actual = kernel(**inputs)
    err = abs(actual - expected) / max(abs(expected), 1e-12)
    print(f"expected={expected:.6f} actual={float(actual):.6f} relerr={err:.3e}")

# revision 25
# speedup vs baseline: 1.1492x; 1.0767x over previous
"""Trainium2 Bass kernel for graph-contrastive loss (nn_PrePrompt_75496935129282).

Computation (reference):
    self = segment_sum(logits_origin, ori_idx, G)       # [G, D]
    pos  = segment_sum(logits_pos,  pos_idx, G)         # [G, D]
    sim[g, k]  = cos(self[g], pos[k])   (eps-guarded norms)
    res[g] = log(sum_s exp(sim[g, neg_idx[g, s]])) - sim[g, g]
    out = mean(res)

Device strategy (8 NeuronCores, SPMD):
  - Nodes sharded 8 ways. Host orders each core's nodes by graph block
    (gid >> 7, 16 blocks of 128) with data-driven per-block chunk
    counts: every 128-node chunk targets one block, so the one-hot
    matmul is [128, 128] per chunk. Blocks are processed EVENS FIRST
    so PSUM/stage halves split by block parity.
  - Streams are fp8e4m3; chunk PAIRS go through one DoubleRow matmul
    (256 nodes per PE pass at 0.5 cycles/row); odd tails use a single
    fp8 matmul. One-hots for 16 chunks are generated with a single
    broadcast is_equal against a block-relative bf16 iota.
  - Exchange (no ring collectives): per-parity AllToAll redistributes
    raw per-core partials (core r receives all 8 cores' partials of
    blocks {2r, 2r+1}); the 8-way sum happens locally (tree adds, f32)
    on gpsimd (pos, hidden under the origin phase) / vector (self).
    Each core normalizes + PE-transposes only its OWN 2 pos blocks and
    a single shared-output AllGather of the 64KB pn_T slice rebuilds
    the full [128d, 2, 2048] column table on every core.
  - Tail: cosine Gram of the core's 256 self rows vs all 2048 columns,
    denominator = exp-accumulate of (sim + ln(count)) with a
    host-precomputed f32 count table, numerator from the raw f32 local
    sums scaled by inverse norms. Per-core losses summed on host.
"""

import os
import sys

sys.path.insert(0, "/opt/trn_rl_repo")

import numpy as np

import concourse.bacc as bacc
import concourse.bass as bass  # noqa: F401
import concourse.mybir as mybir
import concourse.tile as tile
from concourse import bass_isa
from concourse.bass_utils import run_bass_kernel_spmd


def _ensure_ntff_hook():
    """The agent image's antenv lacks axon_hooks; inject it and register
    the ctypes NTFF profiling hook so trace=True works under axon."""
    import types

    import antenv

    if hasattr(antenv, "axon_hooks"):
        return
    mod = types.ModuleType("antenv.axon_hooks")
    mod._hook = None

    def set_axon_ntff_profile_hook(h):
        mod._hook = h

    def get_axon_ntff_profile_hook():
        return mod._hook

    mod.set_axon_ntff_profile_hook = set_axon_ntff_profile_hook
    mod.get_axon_ntff_profile_hook = get_axon_ntff_profile_hook
    sys.modules["antenv.axon_hooks"] = mod
    antenv.axon_hooks = mod
    try:
        from trn_agent_boot.trn_boot import _ntff_profile_via_ctypes

        mod._hook = _ntff_profile_via_ctypes("/opt/axon/libaxon_pjrt.so")
    except Exception as e:  # pragma: no cover
        print(f"ntff hook registration failed: {e}")


F32 = mybir.dt.float32
BF16 = mybir.dt.bfloat16
F8 = mybir.dt.float8e4

G = 2048
S = 127
D = 256
NCORES = 8
P = 128
A = 16  # chunk-count quantum (padding granularity)
AW = 32  # chunks per DMA group (packet size = AW*D fp8 per partition)
NBUK = 16  # graph blocks of 128
GLOC = G // NCORES  # 256

# even blocks first, then odd: PSUM generation k holds blocks ORDER[8k:8k+8]
ORDER = list(range(0, NBUK, 2)) + list(range(1, NBUK, 2))

_MM_RAW = os.environ.get("KERNEL_MM_DT", "f8")
MMDT = BF16 if _MM_RAW == "bf16" else F8


def _chunk_groups(nchunk):
    """DMA groups: small warm-up groups (fast pipeline start), then
    AW-wide groups plus a ragged tail."""
    out = []
    base = 0
    for w in (8, 8, 16):
        if base + w <= nchunk:
            out.append((base, w))
            base += w
    while base < nchunk:
        w = min(AW, nchunk - base)
        out.append((base, w))
        base += w
    return out


def _plan_units(cb):
    """Walk chunks in even-first block order; greedily pair same-bucket
    chunks that sit in the same DMA group (DoubleRow), singles otherwise.

    Returns (units, half_end_unit, nchunk) with units = [(bucket, c0, k)]."""
    nchunk = sum(cb)
    ends = {base + w - 1 for base, w in _chunk_groups(nchunk)}
    units = []
    c = 0
    for b in ORDER:
        rem = cb[b]
        while rem:
            if rem >= 2 and c not in ends and MMDT is F8:
                units.append((b, c, 2))
                c += 2
                rem -= 2
            else:
                units.append((b, c, 1))
                c += 1
                rem -= 1
    assert c == nchunk
    first8 = set(ORDER[:8])
    half_end_unit = max(i for i, u in enumerate(units) if u[0] in first8)
    return units, half_end_unit, nchunk


def build_nc(cb):
    """SPMD Bass program; cb[b] = chunks assigned to graph block b."""
    nchunk = sum(cb)
    assert nchunk % A == 0
    groups32 = _chunk_groups(nchunk)
    ngrp = len(groups32)
    units, half_end_unit, nck = _plan_units(cb)
    assert nck == nchunk
    first_unit = {}
    last_unit = {}
    for i, (b, _, _) in enumerate(units):
        first_unit.setdefault(b, i)
        last_unit[b] = i
    # units grouped by DMA group
    grp_of = {}
    for gi, (base, w) in enumerate(groups32):
        for c in range(base, base + w):
            grp_of[c] = gi
    sup_units = [[] for _ in range(ngrp)]
    for i, (b, c0, k) in enumerate(units):
        sup_units[grp_of[c0]].append((i, b, c0, k))
    slot = {b: ORDER.index(b) % 8 for b in range(NBUK)}

    nc = bacc.Bacc(
        "TRN2",
        target_bir_lowering=False,
        debug=False,
        num_devices=NCORES,
    )
    groups = [list(range(NCORES))]
    EQ = mybir.AluOpType.is_equal
    ADD = mybir.AluOpType.add
    MUL = mybir.AluOpType.mult
    SUB = mybir.AluOpType.subtract
    BYP = mybir.AluOpType.bypass
    AF = mybir.ActivationFunctionType

    # ---- I/O ----
    xp_d = nc.dram_tensor("xp", [nchunk * P * D], MMDT, kind="ExternalInput").ap()
    xo_d = nc.dram_tensor("xo", [nchunk * P * D], MMDT, kind="ExternalInput").ap()
    idx_d = nc.dram_tensor("idx", [P, 2, nchunk], BF16, kind="ExternalInput").ap()
    lnc_d = nc.dram_tensor("lnc", [P, 2, G], F32, kind="ExternalInput").ap()
    loss_out = nc.dram_tensor("loss", [1, 1], F32, kind="ExternalOutput").ap()

    # ---- internal DRAM ----
    # combined stage: ps_stage[c][k] = partial of block 2c+(k&1), k =
    # {0: pos-even, 1: pos-odd, 2: self-even, 3: self-odd}; ONE AllToAll
    # gives ps_recv[q][k] = core q's partial of block 2r+(k&1) (r = me)
    ps_stage = nc.dram_tensor("ps_stage", [NCORES, 4, P, D], F8).ap()
    ps_recv = nc.dram_tensor("ps_recv", [NCORES, 4, P, D], F8).ap()
    # my normalized+transposed pos slice, and the AllGathered full table
    pnT_mine_d = nc.dram_tensor("pnT_mine", [P, 2, 2 * P], F8).ap()
    pnT_all = nc.dram_tensor(
        "pnT_all", [NCORES, P, 2, 2 * P], F8, addr_space="Shared"
    ).ap()
    # warm-up collectives: the first collective after the kernel barrier
    # pays a ~47us channel-setup cost; burn it at t=0 under phase P
    warm_in = nc.dram_tensor("warm_in", [NCORES, 16], F8).ap()
    warm_at = nc.dram_tensor("warm_at", [NCORES, 16], F8).ap()
    warm_ag = nc.dram_tensor(
        "warm_ag", [NCORES, 16], F8, addr_space="Shared"
    ).ap()

    with tile.TileContext(nc) as tc:
        with (
            tc.tile_pool(name="const", bufs=1) as cpool,
            tc.tile_pool(name="big", bufs=1) as big,
        ):
            # warm up the collective channels immediately (no data deps)
            nc.gpsimd.collective_compute(
                "AllToAll", BYP, replica_groups=groups,
                ins=[warm_in], outs=[warm_at],
            )
            nc.gpsimd.collective_compute(
                "AllGather", BYP, replica_groups=groups,
                ins=[warm_in[0]], outs=[warm_ag],
            )

            # ---- constants / one-shot loads ----
            iota_i = cpool.tile([P, P], mybir.dt.int32, tag="iota_i")
            nc.gpsimd.iota(iota_i[:], pattern=[[1, P]], base=0, channel_multiplier=0)
            iota_bf = cpool.tile([P, P], BF16, tag="iota_bf")
            nc.vector.tensor_copy(iota_bf[:], iota_i[:])
            iota_1 = iota_bf[:].rearrange("p (o x) -> p o x", o=1)
            eps_col = cpool.tile([P, 1], F32, tag="eps_col")
            nc.vector.memset(eps_col[:], 1e-16)
            from concourse.masks import make_identity

            ident_f = cpool.tile([P, P], F32, tag="ident_f")
            make_identity(nc, ident_f[:])
            ident = cpool.tile([P, P], BF16, tag="ident")
            nc.vector.tensor_copy(ident[:], ident_f[:])

            it_sb = cpool.tile([P, 2, nchunk], BF16, tag="it")
            nc.sync.dma_start(out=it_sb[:], in_=idx_d)
            lnc_sb = big.tile([P, 2, G], F32, tag="lnc")
            nc.scalar.dma_start(out=lnc_sb[:], in_=lnc_d)

            # ============ segment-sum phase ============
            def seg_phase(x_d, t_row, stage_ap, psum_bufs, tag, on_half, on_full):
                """fp8 DoubleRow bucketed matmuls -> fp8 stage halves.

                stage_ap(par) -> [8, P, D] dram AP for that parity."""
                with (
                    tc.tile_pool(name=f"ps_{tag}", bufs=psum_bufs, space="PSUM") as pseg,
                    tc.tile_pool(name=f"st_{tag}", bufs=3) as stream,
                    tc.tile_pool(name=f"oh_{tag}", bufs=3) as ohp,
                    tc.tile_pool(name=f"sb_{tag}", bufs=2) as segsb,
                ):
                    acc = pseg.tile([P, 8, D], F32, tag="acc")
                    for gi, (base, w) in enumerate(groups32):
                        xt = stream.tile([P, w, D], MMDT, tag=f"xt{w}")
                        # alternate stream groups across two DMA paths
                        (nc.sync if gi % 2 == 0 else nc.gpsimd).dma_start(
                            out=xt[:],
                            in_=x_d[base * P * D : (base + w) * P * D].rearrange(
                                "(p a d) -> p a d", p=P, a=w, d=D
                            ),
                        )
                        ohm = ohp.tile([P, w, P], MMDT, tag=f"ohm{w}")
                        it_b = (
                            it_sb[:, t_row, base : base + w]
                            .rearrange("p (a o) -> p a o", o=1)
                            .broadcast_to([P, w, P])
                        )
                        nc.vector.tensor_tensor(
                            out=ohm[:], in0=iota_1.broadcast_to([P, w, P]),
                            in1=it_b, op=EQ,
                        )
                        for i, b, c0, k in sup_units[gi]:
                            a = c0 - base
                            if k == 2:
                                nc.tensor.matmul(
                                    out=acc[:, slot[b], :],
                                    lhsT=ohm[:, a : a + 2, :],
                                    rhs=xt[:, a : a + 2, :],
                                    start=(i == first_unit[b]),
                                    stop=(i == last_unit[b]),
                                    perf_mode=mybir.MatmulPerfMode.DoubleRow,
                                )
                            else:
                                nc.tensor.matmul(
                                    out=acc[:, slot[b], :],
                                    lhsT=ohm[:, a, :],
                                    rhs=xt[:, a, :],
                                    start=(i == first_unit[b]),
                                    stop=(i == last_unit[b]),
                                )
                            if i == half_end_unit:
                                sbh = segsb.tile([P, 8, D], F8, tag="sbh")
                                nc.scalar.copy(sbh[:], acc[:])
                                nc.scalar.dma_start(
                                    out=stage_ap(0).rearrange("h p d -> p h d"),
                                    in_=sbh[:],
                                )
                                on_half()
                                acc = pseg.tile([P, 8, D], F32, tag="acc")
                    sbh = segsb.tile([P, 8, D], F8, tag="sbh")
                    nc.scalar.copy(sbh[:], acc[:])
                    nc.scalar.dma_start(
                        out=stage_ap(1).rearrange("h p d -> p h d"), in_=sbh[:]
                    )
                    on_full()

            # ---- phase P (pos) then phase O (self); ONE AllToAll at end ----
            seg_phase(
                xp_d, 0, lambda par: ps_stage[:, par], 2, "p",
                on_half=lambda: None,
                on_full=lambda: None,
            )
            seg_phase(
                xo_d, 1, lambda par: ps_stage[:, 2 + par], 2, "o",
                on_half=lambda: None,
                on_full=lambda: nc.gpsimd.collective_compute(
                    "AllToAll",
                    BYP,
                    replica_groups=groups,
                    ins=[ps_stage[:]],
                    outs=[ps_recv[:]],
                ),
            )

            # ================= tail =================
            # one SBUF load of all 32 received partials: [P, core, k*D]
            pr_all = big.tile([P, 8, 4 * D], F8, tag="pr_all")
            nc.gpsimd.dma_start(
                out=pr_all[:].rearrange("p c (k d) -> p c k d", k=4),
                in_=ps_recv.rearrange("c k p d -> p c k d"),
            )
            sums = big.tile([P, 4, D], F32, tag="sums")  # pos-e,pos-o,self-e,self-o
            t4 = big.tile([P, 4, 2 * D], F32, tag="t4")
            t2 = big.tile([P, 2, 2 * D], F32, tag="t2")
            n2 = big.tile([P, 4], F32, tag="n2")
            ln2 = big.tile([P, 4], F32, tag="ln2")
            inv = big.tile([P, 4], F32, tag="inv")
            sq = big.tile([P, 4, D], F32, tag="sq")

            def tree_sum(ks):
                """8-way tree sum over cores of both parities of table ks
                (slice [P, c, 2D] of pr_all) -> sums[:, 2ks:2ks+2]."""
                o = ks * 2 * D
                nc.vector.tensor_tensor(
                    out=t4[:],
                    in0=pr_all[:, 0:4, o : o + 2 * D],
                    in1=pr_all[:, 4:8, o : o + 2 * D],
                    op=ADD,
                )
                nc.vector.tensor_tensor(
                    out=t2[:], in0=t4[:, 0:2], in1=t4[:, 2:4], op=ADD
                )
                nc.vector.tensor_tensor(
                    out=sums[:, 2 * ks : 2 * ks + 2, :].rearrange(
                        "p a d -> p (a d)"
                    ).rearrange("p (o x) -> p o x", o=1),
                    in0=t2[:, 0:1],
                    in1=t2[:, 1:2],
                    op=ADD,
                )

            # pos first (feeds the AllGather), then self during the gather
            pnT_mine = big.tile([P, 2, 2 * P], F8, tag="pnT_mine")
            pn_T = big.tile([P, 2, G], F8, tag="pn_T")
            shat = big.tile([P, 2, D], BF16, tag="shat")
            phn = big.tile([P, 2, D], BF16, tag="phn")
            sn_T = big.tile([P, 2, 2 * P], F8, tag="sn_T")
            with tc.tile_pool(name="ps_tr", bufs=4, space="PSUM") as ptr:
                for ks, norm_dst, trn_src, trn_dst in (
                    (0, phn, phn, pnT_mine),
                    (1, shat, shat, sn_T),
                ):
                    tree_sum(ks)
                    for par in range(2):
                        k = 2 * ks + par
                        nc.scalar.activation(
                            out=sq[:, k, :],
                            in_=sums[:, k, :],
                            func=AF.Square,
                            accum_out=n2[:, k : k + 1],
                        )
                    # rsqrt via exp(-0.5*ln(x)) on the scalar engine
                    nc.scalar.activation(
                        out=ln2[:, 2 * ks : 2 * ks + 2],
                        in_=n2[:, 2 * ks : 2 * ks + 2],
                        func=AF.Ln,
                        bias=eps_col[:],
                    )
                    nc.scalar.activation(
                        out=inv[:, 2 * ks : 2 * ks + 2],
                        in_=ln2[:, 2 * ks : 2 * ks + 2],
                        func=AF.Exp,
                        scale=-0.5,
                    )
                    for par in range(2):
                        k = 2 * ks + par
                        nc.vector.tensor_scalar(
                            out=norm_dst[:, par, :],
                            in0=sums[:, k, :],
                            scalar1=inv[:, k : k + 1],
                            scalar2=None,
                            op0=MUL,
                        )
                    for par in range(2):
                        for db in range(2):
                            tps = ptr.tile([P, P], BF16, tag="tr")
                            nc.tensor.transpose(
                                out=tps[:],
                                in_=trn_src[:, par, db * P : (db + 1) * P],
                                identity=ident[:],
                            )
                            nc.scalar.copy(
                                trn_dst[:, db, par * P : (par + 1) * P], tps[:]
                            )
                    if ks == 0:
                        nc.scalar.dma_start(out=pnT_mine_d, in_=pnT_mine[:])
                        # share my pn_T slice (shared-output AllGather)
                        nc.gpsimd.collective_compute(
                            "AllGather",
                            BYP,
                            replica_groups=groups,
                            ins=[pnT_mine_d],
                            outs=[pnT_all[:]],
                        )
                        # full table: col-block k (128 cols) = graph block k
                        nc.gpsimd.dma_start(
                            out=pn_T[:].rearrange("p h (c x) -> p h c x", c=NCORES),
                            in_=pnT_all.rearrange("c p h x -> p h c x"),
                        )
            # numerator: sim0[p] = sum_par <s_raw, p_raw> * invs * invp
            rd = big.tile([P, 2, D], F32, tag="rd")
            nc.vector.tensor_tensor(
                out=rd[:], in0=sums[:, 2:4, :], in1=sums[:, 0:2, :], op=MUL
            )
            rd2 = big.tile([P, 2], F32, tag="rd2")
            nc.vector.tensor_reduce(
                out=rd2[:], in_=rd[:], axis=mybir.AxisListType.X, op=ADD
            )
            s0a = big.tile([P, 2], F32, tag="s0a")
            nc.vector.tensor_tensor(out=s0a[:], in0=rd2[:], in1=inv[:, 2:4], op=MUL)
            s0b = big.tile([P, 2], F32, tag="s0b")
            nc.vector.tensor_tensor(out=s0b[:], in0=s0a[:], in1=inv[:, 0:2], op=MUL)
            sim0 = big.tile([P, 1], F32, tag="sim0")
            nc.vector.tensor_reduce(
                out=sim0[:], in_=s0b[:], axis=mybir.AxisListType.X, op=ADD
            )

            # ---- Gram + loss: per (row-block lo, column-parity) ----
            denp = big.tile([P, 2], F32, tag="denp")
            with (
                tc.tile_pool(name="ps_gram", bufs=2, space="PSUM") as pgram,
                tc.tile_pool(name="gl", bufs=2) as gl,
            ):
                for lo in range(2):
                    pg = pgram.tile([P, 4, 512], F32, tag="pg")
                    for q in range(4):
                        nc.tensor.matmul(
                            out=pg[:, q, :],
                            lhsT=sn_T[:, :, lo * P : (lo + 1) * P],
                            rhs=pn_T[:, :, q * 512 : (q + 1) * 512],
                            start=True,
                            stop=True,
                            perf_mode=mybir.MatmulPerfMode.DoubleRow,
                        )
                    simln = gl.tile([P, G], F32, tag="simln")
                    nc.vector.tensor_tensor(
                        out=simln[:],
                        in0=pg[:].rearrange("p a b -> p (a b)"),
                        in1=lnc_sb[:, lo, :],
                        op=ADD,
                    )
                    ed = gl.tile([P, G], BF16, tag="ed")
                    nc.scalar.activation(
                        out=ed[:],
                        in_=simln[:],
                        func=AF.Exp,
                        accum_out=denp[:, lo : lo + 1],
                    )

            lden2 = big.tile([P, 2], F32, tag="lden2")
            nc.scalar.activation(out=lden2[:], in_=denp[:], func=AF.Ln)
            t0 = big.tile([P, 1], F32, tag="t0")
            nc.vector.tensor_reduce(
                out=t0[:], in_=lden2[:], axis=mybir.AxisListType.X, op=ADD
            )
            t1 = big.tile([P, 1], F32, tag="t1")
            nc.vector.tensor_tensor(out=t1[:], in0=t0[:], in1=sim0[:], op=SUB)
            ones_col = big.tile([P, 1], F32, tag="ones_col")
            nc.vector.memset(ones_col[:], 1.0)
            with tc.tile_pool(name="ps_ls", bufs=1, space="PSUM") as pls:
                lps = pls.tile([1, 1], F32, tag="lps")
                nc.tensor.matmul(
                    out=lps[:], lhsT=t1[:], rhs=ones_col[:], start=True, stop=True
                )
                lsum1 = big.tile([1, 1], F32, tag="lsum1")
                nc.scalar.copy(lsum1[:], lps[:])
            nc.sync.dma_start(out=loss_out[:], in_=lsum1[:])
    nc.compile()
    return nc


def _chunk_plan(idx_list):
    """cb[b] = chunk count covering max bucket occupancy over all
    (core, table) shards; total padded to a multiple of A."""
    maxc = np.zeros(NBUK, np.int64)
    for gids in idx_list:
        cnt = np.bincount((gids >> 7).astype(np.int64), minlength=NBUK)
        maxc = np.maximum(maxc, cnt)
    cb = [max(1, int(np.ceil(c / P))) for c in maxc]
    i = 0
    while sum(cb) % A != 0:
        cb[i % NBUK] += 1
        i += 1
    return cb


def _pack_shard(x, gids, cb, np_mm):
    """Order a core's nodes bucket-major (even blocks first) into the
    padded chunk layout.

    Returns (x_packed [nsup, P, A, D] np_mm, idx_rel [P, nchunk])."""
    nchunk = sum(cb)
    key = (gids >> 7).astype(np.int64)
    counts = np.bincount(key, minlength=NBUK)
    off = {}
    c = 0
    for b in ORDER:
        off[b] = c * P
        c += cb[b]
    pos_in_order = np.asarray([ORDER.index(b) for b in range(NBUK)], np.int64)
    order = np.argsort(pos_in_order[key], kind="stable")
    dst = np.concatenate([off[b] + np.arange(counts[b]) for b in ORDER])
    xpad = np.zeros((nchunk * P, D), np.float32)
    ipad = np.full((nchunk * P,), -1.0, np.float32)
    xpad[dst] = x[order]
    ipad[dst] = (gids[order] & 127).astype(np.float32)
    blocks = []
    for base, w in _chunk_groups(nchunk):
        blk = xpad[base * P : (base + w) * P].reshape(w, P, D).transpose(1, 0, 2)
        blocks.append(blk.reshape(-1))
    x_packed = np.concatenate(blocks).astype(np_mm)
    idx_rel = np.ascontiguousarray(ipad.reshape(nchunk, P).T)
    return x_packed, idx_rel


def _prep_inputs(logits_origin, logits_pos, ori_idx, pos_idx, neg_idx):
    import ml_dtypes  # noqa: F401

    np_mm = np.dtype(mybir.dt.np(MMDT))
    np_bf = np.dtype(mybir.dt.np(BF16))
    xo = np.ascontiguousarray(np.asarray(logits_origin, dtype=np.float32))
    xp = np.ascontiguousarray(np.asarray(logits_pos, dtype=np.float32))
    oi = np.asarray(ori_idx).astype(np.int64)
    pi = np.asarray(pos_idx).astype(np.int64)
    neg = np.asarray(neg_idx)
    n = xo.shape[0]
    assert xo.shape == (n, D) and xp.shape == (n, D)
    assert neg.shape == (G, S)

    nloc = (n + NCORES - 1) // NCORES
    shards = []
    for r in range(NCORES):
        lo = r * nloc
        hi = min(n, lo + nloc)
        shards.append((xo[lo:hi], oi[lo:hi], xp[lo:hi], pi[lo:hi]))
    cb = _chunk_plan([s[1] for s in shards] + [s[3] for s in shards])

    cnt = np.zeros((G, G), dtype=np.float64)
    rows = np.repeat(np.arange(G), S)
    np.add.at(cnt, (rows, neg.ravel().astype(np.int64)), 1.0)
    with np.errstate(divide="ignore"):
        lncnt = np.where(cnt > 0, np.log(cnt), -30000.0).astype(np.float32)

    in_maps = []
    for r in range(NCORES):
        xo_r, oi_r, xp_r, pi_r = shards[r]
        xp_pk, ip_rel = _pack_shard(xp_r, pi_r, cb, np_mm)
        xo_pk, io_rel = _pack_shard(xo_r, oi_r, cb, np_mm)
        idx_pk = np.stack([ip_rel, io_rel], axis=1).astype(np_bf)  # [P, 2, nchunk]
        # local graphs = blocks {2r, 2r+1} = [256r, 256r+256)
        lnc_r = np.stack(
            [lncnt[r * GLOC + lo * P : r * GLOC + (lo + 1) * P] for lo in range(2)],
            axis=1,
        ).astype(np.float32)  # [P, 2, G]
        in_maps.append(
            {
                "xp": xp_pk,
                "xo": xo_pk,
                "idx": np.ascontiguousarray(idx_pk),
                "lnc": np.ascontiguousarray(lnc_r),
            }
        )
    return cb, in_maps


def kernel(
    logits_origin,
    logits_pos,
    ori_idx,
    pos_idx,
    neg_idx,
    _trace=False,
    _tmpdir=None,
):
    cb, in_maps = _prep_inputs(logits_origin, logits_pos, ori_idx, pos_idx, neg_idx)
    if _trace:
        _ensure_ntff_hook()
    nc = build_nc(cb)
    res = run_bass_kernel_spmd(
        nc,
        in_maps,
        core_ids=list(range(NCORES)),
        trace=_trace,
        tmpdir=_tmpdir,
    )
    kernel._last_results = res
    total = sum(float(res.results[r]["loss"][0, 0]) for r in range(NCORES))
    return np.asarray(np.float32(total / G))


kernel._last_results = None


if __name__ == "__main__":
    rng = np.random.default_rng(0)
    n = 4096
    inputs = {
        "logits_origin": rng.standard_normal((n, D), dtype=np.float32),
        "logits_pos": rng.standard_normal((n, D), dtype=np.float32),
        "ori_idx": rng.integers(0, G, n, dtype=np.int32),
        "pos_idx": rng.integers(0, G, n, dtype=np.int32),
        "neg_idx": rng.integers(0, G, (G, S), dtype=np.int32),
    }

    def np_ref(logits_origin, logits_pos, ori_idx, pos_idx, neg_idx):
        x = logits_origin.astype(np.float64)
        y = logits_pos.astype(np.float64)
        self_l = np.zeros((G, D))
        pos_l = np.zeros((G, D))
        np.add.at(self_l, ori_idx, x)
        np.add.at(pos_l, pos_idx, y)
        eps = 1e-8
        na = np.maximum(np.linalg.norm(self_l, axis=1), eps)
        nb = np.maximum(np.linalg.norm(pos_l, axis=1), eps)
        sh = self_l / na[:, None]
        ph = pos_l / nb[:, None]
        gram = sh @ ph.T
        sim0 = np.einsum("gd,gd->g", sh, ph)
        e = np.exp(gram)
        den = np.array([e[g, neg_idx[g]].sum() for g in range(G)])
        res = np.log(den) - sim0
        return res.mean()

    expected = np_ref(**inputs)
    actual = kernel(**inputs)
    err = abs(actual - expected) / max(abs(expected), 1e-12)
    print(f"expected={expected:.6f} actual={float(actual):.6f} relerr={err:.3e}")